# revision 1
# baseline (speedup 1.0000x reference)
"""Trainium2 Bass kernel for a 2-layer spiking (LIF) MLP scan.

Model (per reference):
  cur1 = x @ W1.T + b1           [B, 25]   (constant over time)
  25 timesteps of:
    reset1 = H(mem1 - 1); mem1 = 0.95*mem1 + cur1 - reset1; spk1 = H(mem1 - 1)
    cur2 = spk1 @ W2.T + b2
    reset2 = H(mem2 - 1); mem2 = 0.95*mem2 + cur2 - reset2; spk2 = H(mem2 - 1)
  outputs: spk2_rec, mem2_rec  each [25, B, 10]

Distribution: pure data parallel over 8 NeuronCores; batch 32768 padded to
32800 = 8 cores x 4100.

Device formulation: doubled units M = 2*mem and sign-coded spikes
sigma = 2*spk - 1 in {-1, +1}:
    M_t = beta*M_{t-1} + chat - sigma_{t-1},   chat = 2*cur - 1
    sigma_t = Sign(M_t - 2)            (ScalarE activation - frees DVE)
  with sigma_init = -1, M_init = 0. Layer-2 drive is a block-diagonal W2
  matmul over sigma1 plus a tiny K=2 matmul of the constant row
  sum_k W2[j,k] + 2*b2[j] - 1 against an all-ones tile. The host halves M2
  and thresholds sigma2 > 0 to recover mem2/spk2 exactly.

Matmul precision: fp16 split-accumulate. x = xa + xb (two fp16 terms holds
~22 of f32's 24 mantissa bits), W = wa + wb likewise; accumulating
xa@wa + xa@wb + xb@wa in fp32 PSUM gives ~f32-class results at 1 cycle/row
(vs 4 cycles/row for native f32 matmul). sigma is exactly representable in
fp16, so the layer-2 matmul needs only the weight split (2 terms).

Per-core layout:
  - x arrives host-transposed as [784, 4100] fp16 split pairs; the layer-1
    bias (2*b1 - 1) is applied as a per-partition ScalarE bias during the
    PSUM->SBUF copy of the GEMM result.
  - Layer-1 state is feature-packed: 5 batch groups x 25 features = 125
    partitions, 820 batch columns -> [125, 820] tiles.
  - Layer-2 state is batch-packed [100, 410]: row 50*h + 10*g + j holds
    feature j of batch element g*820 + h*410 + n.
  - Scheduling: engines run their streams in order, so the batch-half-0
    layer-1 recurrence is emitted as one contiguous block that overlaps the
    half-1 GEMM; a 25-deep sigma ring decouples it from the PE-ordered
    layer-2 matmuls. GpSimd takes a column-split share of the spike
    subtracts in the tail phase.
"""

import numpy as np

BETA = 0.95
T = 25
B_FULL = 32768
D = 784
H1 = 25
H2 = 10
N_CORES = 8
BC = 4100          # per-core batch (padded)
G = 5              # feature-packing groups
COLS = BC // G     # 820 batch columns per group
HALF = COLS // 2   # 410
P1 = G * H1        # 125 partitions for layer-1 state
P2 = 2 * G * H2    # 100 rows of the layer-2 tile

KC = 7           # k-chunks of 112 rows: 7*112 = 784
KSZ = D // KC

# columns (of each 410-col half) whose layer-1 spike-subtract runs on GpSimd
POOL_HALF = 288
SIG_RING = 25      # sigma1 tiles: one per step (full cross-step pipelining)
L2_POOL = 300      # layer-2 subtract columns handled by gpsimd

GEMM_MODE = "f16x3"    # 'f32' | 'f16x3'
SCANMM_MODE = "f16x2"  # 'f32' | 'f16x2'

_CACHED = {}


def _build_program(variant="v3"):
    from contextlib import ExitStack

    import concourse.bacc as bacc
    import concourse.tile as tile
    from concourse import mybir

    dt = mybir.dt
    alu = mybir.AluOpType
    act_fn = mybir.ActivationFunctionType

    nc = bacc.Bacc(
        "TRN2",
        target_bir_lowering=False,
        debug=False,
        enable_asserts=False,
        num_devices=N_CORES,
    )

    f16 = dt.float16
    f32 = dt.float32

    if GEMM_MODE == "f16x3":
        xa_d = nc.dram_tensor("xa", [D, BC], f16, kind="ExternalInput").ap()
        xb_d = nc.dram_tensor("xb", [D, BC], f16, kind="ExternalInput").ap()
        w1a_d = nc.dram_tensor("w1a", [D, G * P1], f16, kind="ExternalInput").ap()
        w1b_d = nc.dram_tensor("w1b", [D, G * P1], f16, kind="ExternalInput").ap()
    else:
        xT_d = nc.dram_tensor("xT", [D, BC], f32, kind="ExternalInput").ap()
        w1t_d = nc.dram_tensor("w1t", [D, G * P1], f32, kind="ExternalInput").ap()

    smm_dt = f16 if SCANMM_MODE == "f16x2" else f32
    # per-partition GEMM bias (2*b1[j] - 1 at row 25g+j)
    cb1_d = nc.dram_tensor("cb1", [P1, 1], f32, kind="ExternalInput").ap()
    # the layer-2 constant (sum_k W2 + 2*b2 - 1) enters via a tiny K=2 matmul
    # of split bias rows against a constant ones tile
    ones2_d = nc.dram_tensor("ones2", [2, HALF], smm_dt, kind="ExternalInput").ap()
    wbdc_d = nc.dram_tensor("wbdc", [2, P2], smm_dt, kind="ExternalInput").ap()
    n_wbd = 4 if SCANMM_MODE == "f16x2" else 2
    wbd_d = nc.dram_tensor(
        "wbd", [P1, n_wbd * P2], smm_dt, kind="ExternalInput"
    ).ap()

    # merged per-step record: cols 0:410 = M2, cols 410:820 = sigma2
    rec_out = nc.dram_tensor(
        "rec_out", [T, P2, COLS], f32, kind="ExternalOutput"
    ).ap()

    with tile.TileContext(nc) as tc, ExitStack() as ctx:
        w1_pool = ctx.enter_context(tc.tile_pool(name="w1", bufs=1))
        wbd_pool = ctx.enter_context(tc.tile_pool(name="wbd", bufs=1))
        xin_pool = ctx.enter_context(tc.tile_pool(name="xin", bufs=6))
        state_pool = ctx.enter_context(tc.tile_pool(name="state", bufs=1))
        l2_pool = ctx.enter_context(tc.tile_pool(name="l2", bufs=6))
        psA_pool = ctx.enter_context(tc.tile_pool(name="psA", bufs=2, space="PSUM"))
        ps2_pool = ctx.enter_context(tc.tile_pool(name="ps2", bufs=5, space="PSUM"))

        # --- weights: one [112, 7*625] tile per split term ---
        w1_tiles = []
        w1_srcs = (
            [(w1a_d, "w1a"), (w1b_d, "w1b")]
            if GEMM_MODE == "f16x3"
            else [(w1t_d, "w1t")]
        )
        w1dt = f16 if GEMM_MODE == "f16x3" else f32
        # per-chunk DMAs so the first matmuls can start as early as possible
        for src_d, tag in w1_srcs:
            wt = w1_pool.tile([KSZ, KC * G * P1], w1dt, tag=tag)
            w1_tiles.append(wt)
        # weights ride the ScalarE HWDGE queue so the first x tiles aren't
        # stuck behind them on the sync queue
        for k in range(KC):
            for (src_d, tag), wt in zip(w1_srcs, w1_tiles):
                nc.scalar.dma_start(
                    wt[:, k * G * P1 : (k + 1) * G * P1],
                    src_d[k * KSZ : (k + 1) * KSZ, :],
                )

        wbd_tiles = []
        for i in range(n_wbd):
            wt = wbd_pool.tile([P1, P2], smm_dt, tag=f"wbd{i}")
            nc.scalar.dma_start(wt[:], wbd_d[:, i * P2 : (i + 1) * P2])
            wbd_tiles.append(wt)
        wbdc = wbd_pool.tile([2, P2], smm_dt, tag="wbdc")
        nc.scalar.dma_start(wbdc[:], wbdc_d[:])
        ones2 = wbd_pool.tile([2, HALF], smm_dt, tag="ones2")
        nc.scalar.dma_start(ones2[:], ones2_d[:])

        # --- persistent state ---
        chat1 = state_pool.tile([P1, COLS], f32, tag="chat1")
        mh1A = state_pool.tile([P1, COLS], f32, tag="mh1A")
        mh1B = state_pool.tile([P1, COLS], f32, tag="mh1B")
        # sigma ring: one tile per step (written fully by Sign before reads)
        sig_ring = []
        for i in range(SIG_RING):
            sg = state_pool.tile([P1, COLS], smm_dt, tag=f"sig{i}")
            sig_ring.append(sg)
        sig_init = state_pool.tile([P1, COLS], smm_dt, tag="sig_init")
        nc.vector.memset(sig_init[:], -1.0)
        nc.vector.memset(mh1B[:], 0.0)
        cb1 = state_pool.tile([P1, 1], f32, tag="cb1")
        nc.sync.dma_start(cb1[:], cb1_d[:])

        rec0 = l2_pool.tile([P2, COLS], f32, tag="rec")
        nc.vector.memset(rec0[:], 0.0)
        nc.vector.memset(rec0[:, HALF:COLS], -1.0)
        mh2_prev = rec0[:, 0:HALF]
        s2_prev = rec0[:, HALF:COLS]

        # per-partition bias column (-2.0) for the Sign activations
        biasc = state_pool.tile([128, 1], f32, tag="biasc")
        nc.vector.memset(biasc[:], -2.0)

        # --- main GEMM -> chat1 = 2*cur1 - 1, feature-packed [125, 820] ---
        # One k-blocked DMA per (g, h) per x-term: [112, 7*410] tiles.
        xdt = f16 if GEMM_MODE == "f16x3" else f32
        x_srcs = [xa_d, xb_d] if GEMM_MODE == "f16x3" else [xT_d]
        for h in range(2):
            ps = psA_pool.tile([P1, HALF], f32)
            first = True
            for g in range(G):
                col0 = g * COLS + h * HALF
                xts = []
                for xi, src_d in enumerate(x_srcs):
                    xt = xin_pool.tile([KSZ, KC * HALF], xdt, tag=f"x{xi}")
                    nc.sync.dma_start(
                        xt[:].rearrange("p (c n) -> p c n", c=KC),
                        src_d.rearrange("(c p) n -> p c n", p=KSZ)[
                            :, :, col0 : col0 + HALF
                        ],
                    )
                    xts.append(xt)
                if GEMM_MODE == "f16x3":
                    terms = [(0, 0), (1, 0), (0, 1)]  # (w term, x term)
                else:
                    terms = [(0, 0)]
                for k in range(KC):
                    last_k = g == G - 1 and k == KC - 1
                    for j, (wi, xi) in enumerate(terms):
                        nc.tensor.matmul(
                            ps[:],
                            lhsT=w1_tiles[wi][
                                :, k * G * P1 + g * P1 : k * G * P1 + (g + 1) * P1
                            ],
                            rhs=xts[xi][:, k * HALF : (k + 1) * HALF],
                            start=first,
                            stop=last_k and j == len(terms) - 1,
                        )
                        first = False
            # bias (2*b1 - 1) folded into the PSUM->SBUF copy
            nc.scalar.activation(
                chat1[:, h * HALF : (h + 1) * HALF], ps[:],
                act_fn.Identity, bias=cb1[:], scale=1.0,
            )

        # --- the 25-step scan ---
        # Engines execute in order, so the h0 layer-1 recurrence is emitted as
        # one contiguous block: it only needs the GEMM's h0 output and can run
        # 25 steps deep while the h1 GEMM is still streaming. The h1 block and
        # the (batch-complete) layer-2 chain follow.
        def l1_step(t, h, pool_cols):
            prev = sig_ring[t - 1] if t > 0 else sig_init
            new = sig_ring[t]
            m_prev = mh1B if t % 2 == 0 else mh1A
            m_cur = mh1A if t % 2 == 0 else mh1B
            sl = slice(h * HALF, (h + 1) * HALF)
            nc.vector.scalar_tensor_tensor(
                m_cur[:, sl], m_prev[:, sl], BETA, chat1[:, sl],
                op0=alu.mult, op1=alu.add,
            )
            if pool_cols:
                pc = slice(h * HALF, h * HALF + pool_cols)
                nc.gpsimd.tensor_tensor(
                    m_cur[:, pc], m_cur[:, pc], prev[:, pc], op=alu.subtract
                )
            vc = slice(h * HALF + pool_cols, (h + 1) * HALF)
            nc.vector.tensor_tensor(
                m_cur[:, vc], m_cur[:, vc], prev[:, vc], op=alu.subtract
            )
            # sigma1 = Sign(M1 - 2)
            nc.scalar.activation(
                new[:, sl], m_cur[:, sl], act_fn.Sign,
                bias=biasc[0:P1, :], scale=1.0,
            )

        # h0 front-run: subtract stays on DVE (in-order, no cross-engine hop
        # in the recurrence) so the chain paces with the h1 GEMM stream
        for t in range(T):
            l1_step(t, 0, 0)

        mh2_prev = rec0[:, 0:HALF]
        s2_prev = rec0[:, HALF:COLS]
        for t in range(T):
            l1_step(t, 1, POOL_HALF)
            new = sig_ring[t]
            # layer-2 drive [100, 410]: bias matmul + blockdiag terms
            ps2 = ps2_pool.tile([P2, HALF], f32)
            if SCANMM_MODE == "f16x2":
                mms = [(0, 0), (1, 0), (2, 1), (3, 1)]  # (wbd idx, half)
            else:
                mms = [(0, 0), (1, 1)]
            nc.tensor.matmul(
                ps2[:], lhsT=wbdc[:], rhs=ones2[:], start=True, stop=False
            )
            for j, (wi, h) in enumerate(mms):
                nc.tensor.matmul(
                    ps2[:],
                    lhsT=wbd_tiles[wi][:],
                    rhs=new[:, h * HALF : (h + 1) * HALF],
                    start=False,
                    stop=(j == len(mms) - 1),
                )
            # M2 = beta*M2_prev + chat2; record tile holds [M2 | sigma2]
            rec = l2_pool.tile([P2, COLS], f32, tag="rec")
            mh2n = rec[:, 0:HALF]
            s2n = rec[:, HALF:COLS]
            nc.vector.scalar_tensor_tensor(
                mh2n, mh2_prev, BETA, ps2[:], op0=alu.mult, op1=alu.add
            )
            # M2 -= sigma2_prev (split pool/dve)
            nc.gpsimd.tensor_tensor(
                rec[:, 0:L2_POOL], rec[:, 0:L2_POOL],
                s2_prev[:, 0:L2_POOL], op=alu.subtract,
            )
            nc.vector.tensor_tensor(
                rec[:, L2_POOL:HALF], rec[:, L2_POOL:HALF],
                s2_prev[:, L2_POOL - HALF :], op=alu.subtract,
            )
            # sigma2 = Sign(M2 - 2)
            nc.scalar.activation(
                s2n, mh2n, act_fn.Sign, bias=biasc[0:P2, :], scale=1.0
            )
            nc.sync.dma_start(rec_out[t], rec[:])
            mh2_prev = mh2n
            s2_prev = s2n

    nc.compile()
    return nc


def _get_nc(variant="v3"):
    key = (variant, GEMM_MODE, SCANMM_MODE)
    if key not in _CACHED:
        _CACHED[key] = _build_program(variant)
    return _CACHED[key]


def _f16_split(a):
    hi = a.astype(np.float16)
    lo = (a.astype(np.float32) - hi.astype(np.float32)).astype(np.float16)
    return hi, lo


def _host_inputs(x, W1, b1, W2, b2):
    ins = {}
    xp = np.zeros((D, N_CORES * BC), np.float32)
    xp[:, : x.shape[0]] = x.T
    # chat1 = x @ (2*W1).T + (2*b1 - 1): block-column layout; the bias part
    # is applied on-device via the per-partition cb1 column.
    w1blocks = np.zeros((D, G * P1), np.float32)
    for g in range(G):
        w1blocks[:, P1 * g + H1 * g : P1 * g + H1 * (g + 1)] = 2.0 * W1.T
    ins["cb1"] = np.tile(2.0 * b1 - 1.0, G).astype(np.float32)[:, None]
    if GEMM_MODE == "f16x3":
        ins["xa"], ins["xb"] = _f16_split(xp)
        ins["w1a"], ins["w1b"] = _f16_split(w1blocks)
    else:
        ins["xT"] = xp
        ins["w1t"] = w1blocks
    # chat2 = sigma1 @ blockdiag(W2.T) + (sum_k W2[j,k] + 2*b2[j] - 1)
    bias2 = (W2.sum(axis=1) + 2.0 * b2 - 1.0).astype(np.float32)
    wbdf = np.zeros((P1, 2 * P2), np.float32)
    for blk, h in ((0, 0), (1, 1)):
        off = blk * P2 + h * G * H2
        for g in range(G):
            wbdf[g * H1 : (g + 1) * H1, off + g * H2 : off + (g + 1) * H2] = W2.T
    # bias_row[0, 50h+10g+j] = bias2[j]
    bias_row = np.tile(bias2, 2 * G)[None, :]
    smm_np = np.float16 if SCANMM_MODE == "f16x2" else np.float32
    ins["ones2"] = np.ones((2, HALF), smm_np)
    if SCANMM_MODE == "f16x2":
        A, B = wbdf[:, :P2], wbdf[:, P2:]
        Aa, Ab = _f16_split(A)
        Ba, Bb = _f16_split(B)
        # tile order [0..3] = Aa, Ab, Ba, Bb pairs with batch halves 0,0,1,1
        ins["wbd"] = np.concatenate([Aa, Ab, Ba, Bb], axis=1)
        Ca, Cb = _f16_split(bias_row)
        ins["wbdc"] = np.concatenate([Ca, Cb], axis=0)
    else:
        ins["wbd"] = wbdf
        ins["wbdc"] = np.concatenate([bias_row, np.zeros_like(bias_row)], axis=0)
    return ins


def kernel(x, W1, b1, W2, b2, _variant="v3", _trace=False, _tmpdir=None):
    from concourse.bass_utils import run_bass_kernel_spmd

    x = np.asarray(x, np.float32)
    W1 = np.asarray(W1, np.float32)
    b1 = np.asarray(b1, np.float32)
    W2 = np.asarray(W2, np.float32)
    b2 = np.asarray(b2, np.float32)
    B = x.shape[0]
    assert B == B_FULL, f"kernel hardcoded for B={B_FULL}, got {B}"

    full = _host_inputs(x, W1, b1, W2, b2)
    percore = [k for k in full if k in ("xa", "xb", "xT")]
    shared = {k: v for k, v in full.items() if k not in percore}
    in_maps = []
    for c in range(N_CORES):
        m = dict(shared)
        for k in percore:
            m[k] = np.ascontiguousarray(full[k][:, c * BC : (c + 1) * BC])
        in_maps.append(m)

    nc = _get_nc(_variant)
    res = run_bass_kernel_spmd(
        nc,
        in_maps,
        core_ids=list(range(N_CORES)),
        trace=_trace,
        tmpdir=_tmpdir,
    )

    spk = np.empty((T, N_CORES * BC, H2), np.float32)
    mem = np.empty((T, N_CORES * BC, H2), np.float32)
    for c in range(N_CORES):
        r = res.results[c]["rec_out"]  # [T, 100, 820]: [M2 | sigma2]
        for ci, dst in ((0, mem), (1, spk)):
            q = r[:, :, ci * HALF : (ci + 1) * HALF]
            q = q.reshape(T, 2, G, H2, HALF)  # [t, h, g, j, n]
            q = q.transpose(0, 2, 1, 4, 3)  # [t, g, h, n, j]
            dst[:, c * BC : (c + 1) * BC, :] = q.reshape(T, BC, H2)
    spk = (spk[:, :B_FULL, :] > 0.0).astype(np.float32)
    mem = mem[:, :B_FULL, :] * np.float32(0.5)
    kernel._last_results = res
    return spk, mem



# revision 31
# speedup vs baseline: 1.0118x; 1.0118x over previous
"""Trainium2 Bass kernel for a 2-layer spiking (LIF) MLP scan.

Model (per reference):
  cur1 = x @ W1.T + b1           [B, 25]   (constant over time)
  25 timesteps of:
    reset1 = H(mem1 - 1); mem1 = 0.95*mem1 + cur1 - reset1; spk1 = H(mem1 - 1)
    cur2 = spk1 @ W2.T + b2
    reset2 = H(mem2 - 1); mem2 = 0.95*mem2 + cur2 - reset2; spk2 = H(mem2 - 1)
  outputs: spk2_rec, mem2_rec  each [25, B, 10]

Distribution: pure data parallel over 8 NeuronCores; batch 32768 padded to
32800 = 8 cores x 4100.

Device formulation: doubled units M = 2*mem; layer-1 spikes sign-coded
sigma1 = 2*spk1 - 1 (ScalarE Sign, f16, feeds the W2 matmuls); layer-2
spikes 0/1-coded s2 = (M2' > 2 - D_t) via DVE is_gt:
    M1_t = beta*M1_{t-1} + chat - sigma1_{t-1},  chat = 2*cur1 - 1
    M2'_t = beta*M2'_{t-1} + (sigma1_t @ W2bd - 2*s2_{t-1})   [PSUM accum]
The layer-2 constant drive c = sum_k W2[j,k] + 2*b2[j] is removed from the
device recurrence; its accumulation D_t = beta*D_{t-1} + c is a host-side
per-(partition, step) table that shifts the is_gt threshold (2 - D_t). The
s2 reset-subtract is a PE matmul accumulate (lhsT = -2I) into the same
PSUM tile as the W2 drive. The raw f32 M2' ring is DMA'd out directly; the
host recovers mem2 = 0.5*(M2' + D_t) and spk2 = (s2 > 0.5).

Matmul precision: fp16 split-accumulate. x = xa + xb (two fp16 terms hold
~22 of f32's 24 mantissa bits), W = wa + wb likewise; accumulating
xa@wa + xa@wb + xb@wa in fp32 PSUM gives ~f32-class results at 1 cycle/row.
Spikes are exactly representable in fp16, so the layer-2 matmuls need only
the weight split (2 terms per batch half).

Per-core layout:
  - x arrives host-transposed as [784, 4100] fp16 split pairs; W1 is kept
    COMPACT: a zero-padded sliding-window strip [112, 7*245] per term, so
    each feature-packing group g gets a 125-wide lhsT window that places
    W1 at output partitions 25g:25(g+1) (PSUM base partition stays 0).
  - Layer-1 state is feature-packed: 5 batch groups x 25 features = 125
    partitions, 820 batch columns -> [125, 820] tiles.
  - Layer-2 state is batch-packed [100, 410]: row 50*h + 10*g + j holds
    feature j of batch element g*820 + h*410 + n.
  - The layer-1 recurrence runs as independent column chains: a DVE chain
    (stt + subtract, in-order on DVE) and a Pool chain (both ops in
    TensorScalarPtr form for gpsimd's better efficiency class). The Sign
    lives on ScalarE with one step of slack.
  - Emission order: GEMM-h0 -> h0 layer-1 front-run -> GEMM-h1 -> main
    loop, so ScalarE's h0 Signs are not queued behind the h1 PSUM copy
    (engines execute their streams in order).
"""

import numpy as np

BETA = 0.95
T = 25
B_FULL = 32768
D = 784
H1 = 25
H2 = 10
N_CORES = 8
BC = 4100          # per-core batch (padded)
G = 5              # feature-packing groups
COLS = BC // G     # 820 batch columns per group
HALF = COLS // 2   # 410
P1 = G * H1        # 125 partitions for layer-1 state
P2 = 2 * G * H2    # 100 rows of the layer-2 tile

KC = 7           # k-chunks of 112 rows: 7*112 = 784
KSZ = D // KC

# engine split knobs (tuned against TimelineSim)
POOL_H0 = 0        # h0 layer-1 subtract cols on gpsimd (rest DVE)
POOL_H1 = 288      # h1 layer-1 subtract cols on gpsimd (rest DVE)
SIG_RING = 25      # sigma1 tiles: one per step (full cross-step pipelining)
MH2_RING = 10      # layer-2 state/spike ring depth

_CACHED = {}


def _build_program(variant="v5"):
    from contextlib import ExitStack

    import concourse.bacc as bacc
    import concourse.tile as tile
    from concourse import mybir

    dt = mybir.dt
    alu = mybir.AluOpType
    act_fn = mybir.ActivationFunctionType

    nc = bacc.Bacc(
        "TRN2",
        target_bir_lowering=False,
        debug=False,
        enable_asserts=False,
        num_devices=N_CORES,
    )

    f16 = dt.float16
    f32 = dt.float32

    xa_d = nc.dram_tensor("xa", [D, BC], f16, kind="ExternalInput").ap()
    xb_d = nc.dram_tensor("xb", [D, BC], f16, kind="ExternalInput").ap()
    w1a_d = nc.dram_tensor("w1a", [D, H1], f16, kind="ExternalInput").ap()
    w1b_d = nc.dram_tensor("w1b", [D, H1], f16, kind="ExternalInput").ap()
    cb1_d = nc.dram_tensor("cb1", [P1, 1], f32, kind="ExternalInput").ap()
    # 4 block-diagonal W2 terms: [Aa, Ab, Ba, Bb] pairs with halves 0,0,1,1
    wbd_d = nc.dram_tensor("wbd", [P1, 4 * P2], f16, kind="ExternalInput").ap()
    negi_d = nc.dram_tensor("negi", [P2, P2], f16, kind="ExternalInput").ap()
    # per-step layer-2 threshold: 2 - D_t per partition
    t2tab_d = nc.dram_tensor("t2tab", [P2, T], f32, kind="ExternalInput").ap()

    mem_out = nc.dram_tensor(
        "mem_out", [T, P2, HALF], f32, kind="ExternalOutput"
    ).ap()
    sig_out = nc.dram_tensor(
        "sig_out", [T, P2, HALF], f16, kind="ExternalOutput"
    ).ap()

    with tile.TileContext(nc) as tc, ExitStack() as ctx:
        w1_pool = ctx.enter_context(tc.tile_pool(name="w1", bufs=1))
        wbd_pool = ctx.enter_context(tc.tile_pool(name="wbd", bufs=1))
        xin_pool = ctx.enter_context(tc.tile_pool(name="xin", bufs=6))
        state_pool = ctx.enter_context(tc.tile_pool(name="state", bufs=1))
        psA_pool = ctx.enter_context(tc.tile_pool(name="psA", bufs=2, space="PSUM"))
        ps2_pool = ctx.enter_context(tc.tile_pool(name="ps2", bufs=5, space="PSUM"))

        # --- weights: zero-padded sliding-window strip per split term ---
        WSTRIP = P1 + 120  # 245
        w1_tiles = []
        for src_d, tag in ((w1a_d, "w1a"), (w1b_d, "w1b")):
            wt = w1_pool.tile([KSZ, KC * WSTRIP], f16, tag=tag)
            nc.vector.memset(wt[:], 0.0)
            nc.scalar.dma_start(
                wt[:].rearrange("p (c w) -> p c w", c=KC)[:, :, 120 : 120 + H1],
                src_d.rearrange("(c p) n -> p c n", p=KSZ),
            )
            w1_tiles.append(wt)

        wbd_t = wbd_pool.tile([P1, 4 * P2], f16, tag="wbd")
        nc.scalar.dma_start(wbd_t[:], wbd_d[:])
        wbd_tiles = [wbd_t[:, i * P2 : (i + 1) * P2] for i in range(4)]
        negi = wbd_pool.tile([P2, P2], f16, tag="negi")
        nc.scalar.dma_start(negi[:], negi_d[:])
        t2tab = wbd_pool.tile([P2, T], f32, tag="t2tab")
        nc.scalar.dma_start(t2tab[:], t2tab_d[:])

        # --- persistent state ---
        chat1 = state_pool.tile([P1, COLS], f32, tag="chat1")
        mh1A = state_pool.tile([P1, COLS], f32, tag="mh1A")
        mh1B = state_pool.tile([P1, COLS], f32, tag="mh1B")
        sig_ring = []
        for i in range(SIG_RING):
            sg = state_pool.tile([P1, COLS], f16, tag=f"sig{i}")
            sig_ring.append(sg)
        sig_init = state_pool.tile([P1, COLS], f16, tag="sig_init")
        nc.vector.memset(sig_init[:], -1.0)
        nc.vector.memset(mh1B[:], 0.0)
        cb1 = state_pool.tile([P1, 1], f32, tag="cb1")
        nc.sync.dma_start(cb1[:], cb1_d[:])

        # layer-2 state/spike rings: one tile per slot
        mh2_ring = []
        s2_ring = []
        for i in range(MH2_RING):
            mh2slot = state_pool.tile([P2, HALF], f32, tag=f"mh2_{i}")
            mh2_ring.append(mh2slot[:])
            s2slot = state_pool.tile([P2, HALF], f16, tag=f"s2_{i}")
            s2_ring.append(s2slot[:])
        mh2_init = state_pool.tile([P2, HALF], f32, tag="mh2i")
        nc.vector.memset(mh2_init[:], 0.0)
        s2_init = state_pool.tile([P2, HALF], f16, tag="s2i")
        nc.vector.memset(s2_init[:], 0.0)

        # per-partition bias column (-2.0) for the layer-1 Sign activations
        biasc = state_pool.tile([128, 1], f32, tag="biasc")
        nc.vector.memset(biasc[:], -2.0)

        # --- main GEMM -> chat1 = 2*cur1 - 1, feature-packed [125, 820] ---
        def gemm_half(h):
            ps = psA_pool.tile([P1, HALF], f32)
            for g in range(G):
                col0 = g * COLS + h * HALF
                xts = []
                for xi, src_d in enumerate((xa_d, xb_d)):
                    xt = xin_pool.tile([KSZ, KC * HALF], f16, tag=f"x{xi}")
                    nc.sync.dma_start(
                        xt[:].rearrange("p (c n) -> p c n", c=KC),
                        src_d.rearrange("(c p) n -> p c n", p=KSZ)[
                            :, :, col0 : col0 + HALF
                        ],
                    )
                    xts.append(xt)
                terms = [(0, 0), (1, 0), (0, 1)]  # (w term, x term)
                for k in range(KC):
                    for j, (wi, xi) in enumerate(terms):
                        off = k * WSTRIP + 120 - H1 * g
                        nc.tensor.matmul(
                            ps[:],
                            lhsT=w1_tiles[wi][:, off : off + P1],
                            rhs=xts[xi][:, k * HALF : (k + 1) * HALF],
                            start=(g == 0 and k == 0 and j == 0),
                            stop=(g == G - 1 and k == KC - 1
                                  and j == len(terms) - 1),
                        )
            # bias (2*b1 - 1) folded into the PSUM->SBUF copy
            nc.scalar.activation(
                chat1[:, h * HALF : (h + 1) * HALF], ps[:],
                act_fn.Identity, bias=cb1[:], scale=1.0,
            )

        # --- the 25-step scan ---
        def l1_step(t, h, pool_cols):
            # stt on DVE (gpsimd cannot run scalar_tensor_tensor); the
            # sigma-subtract splits column-wise between DVE (in-order with
            # the stt) and Pool (tensor_tensor, the only gpsimd ALU form
            # walrus accepts)
            prev = sig_ring[t - 1] if t > 0 else sig_init
            new = sig_ring[t]
            m_prev = mh1B if t % 2 == 0 else mh1A
            m_cur = mh1A if t % 2 == 0 else mh1B
            sl = slice(h * HALF, (h + 1) * HALF)
            nc.vector.scalar_tensor_tensor(
                m_cur[:, sl], m_prev[:, sl], BETA, chat1[:, sl],
                op0=alu.mult, op1=alu.add,
            )
            dc = slice(h * HALF, (h + 1) * HALF - pool_cols)
            nc.vector.tensor_tensor(
                m_cur[:, dc], m_cur[:, dc], prev[:, dc], op=alu.subtract
            )
            if pool_cols:
                pc = slice((h + 1) * HALF - pool_cols, (h + 1) * HALF)
                nc.gpsimd.tensor_tensor(
                    m_cur[:, pc], m_cur[:, pc], prev[:, pc], op=alu.subtract
                )
            # sigma1 = Sign(M1 - 2)
            nc.scalar.activation(
                new[:, sl], m_cur[:, sl], act_fn.Sign,
                bias=biasc[0:P1, :], scale=1.0,
            )

        gemm_half(0)
        for t in range(T):
            l1_step(t, 0, POOL_H0)
        gemm_half(1)

        mh2_prev = mh2_init[:]
        s2_prev = s2_init[:]
        for t in range(T):
            l1_step(t, 1, POOL_H1)
            new = sig_ring[t]
            # layer-2 drive [100, 410]: blockdiag W2 terms + (-2I) s2 reset
            ps2 = ps2_pool.tile([P2, HALF], f32)
            mms = [(0, 0), (1, 0), (2, 1), (3, 1)]  # (wbd idx, half)
            for j, (wi, h) in enumerate(mms):
                nc.tensor.matmul(
                    ps2[:],
                    lhsT=wbd_tiles[wi],
                    rhs=new[:, h * HALF : (h + 1) * HALF],
                    start=(j == 0),
                    stop=False,
                )
            nc.tensor.matmul(
                ps2[:], lhsT=negi[:], rhs=s2_prev, start=False, stop=True
            )
            # M2' = beta*M2'_prev + (drive - 2*s2_prev)
            mh2n = mh2_ring[t % MH2_RING]
            nc.vector.scalar_tensor_tensor(
                mh2n, mh2_prev, BETA, ps2[:], op0=alu.mult, op1=alu.add
            )
            # s2 = (M2' > 2 - D_t)   (f16 0/1)
            s2n = s2_ring[t % MH2_RING]
            nc.vector.tensor_scalar(
                s2n, mh2n, t2tab[:, t : t + 1], None, op0=alu.is_gt
            )
            nc.sync.dma_start(mem_out[t], mh2n)
            nc.sync.dma_start(sig_out[t], s2n)
            mh2_prev = mh2n
            s2_prev = s2n

    nc.compile()
    return nc


def _get_nc(variant="v5"):
    key = (variant,)
    if key not in _CACHED:
        _CACHED[key] = _build_program(variant)
    return _CACHED[key]


def _f16_split(a):
    hi = a.astype(np.float16)
    lo = (a.astype(np.float32) - hi.astype(np.float32)).astype(np.float16)
    return hi, lo


def _d_table(W2, b2):
    # D_t = beta*D_{t-1} + c, c = sum_k W2[j,k] + 2*b2[j] per row j
    c = (W2.sum(axis=1) + 2.0 * b2).astype(np.float64)
    crow = np.tile(c, 2 * G)  # [P2] per partition 50h+10g+j
    Dt = np.zeros((P2, T), np.float64)
    acc = np.zeros(P2, np.float64)
    for t in range(T):
        acc = BETA * acc + crow
        Dt[:, t] = acc
    return Dt


def _host_inputs(x, W1, b1, W2, b2):
    ins = {}
    xp = np.zeros((D, N_CORES * BC), np.float32)
    xp[:, : x.shape[0]] = x.T
    ins["xa"], ins["xb"] = _f16_split(xp)
    w1t = 2.0 * W1.T.astype(np.float32)
    ins["w1a"], ins["w1b"] = _f16_split(w1t)
    ins["cb1"] = np.tile(2.0 * b1 - 1.0, G).astype(np.float32)[:, None]

    # chat2 = sigma1 @ blockdiag(W2.T); constant part goes via the D table
    wbdf = np.zeros((P1, 2 * P2), np.float32)
    for blk, h in ((0, 0), (1, 1)):
        off = blk * P2 + h * G * H2
        for g in range(G):
            wbdf[g * H1 : (g + 1) * H1, off + g * H2 : off + (g + 1) * H2] = W2.T
    A, B = wbdf[:, :P2], wbdf[:, P2:]
    Aa, Ab = _f16_split(A)
    Ba, Bb = _f16_split(B)
    ins["wbd"] = np.concatenate([Aa, Ab, Ba, Bb], axis=1)
    ins["negi"] = (-2.0 * np.eye(P2)).astype(np.float16)
    ins["t2tab"] = (2.0 - _d_table(W2, b2)).astype(np.float32)
    return ins


def kernel(x, W1, b1, W2, b2, _variant="v5", _trace=False, _tmpdir=None):
    from concourse.bass_utils import run_bass_kernel_spmd

    x = np.asarray(x, np.float32)
    W1 = np.asarray(W1, np.float32)
    b1 = np.asarray(b1, np.float32)
    W2 = np.asarray(W2, np.float32)
    b2 = np.asarray(b2, np.float32)
    B = x.shape[0]
    assert B == B_FULL, f"kernel hardcoded for B={B_FULL}, got {B}"

    full = _host_inputs(x, W1, b1, W2, b2)
    percore = ("xa", "xb")
    shared = {k: v for k, v in full.items() if k not in percore}
    in_maps = []
    for c in range(N_CORES):
        m = dict(shared)
        for k in percore:
            m[k] = np.ascontiguousarray(full[k][:, c * BC : (c + 1) * BC])
        in_maps.append(m)

    nc = _get_nc(_variant)
    res = run_bass_kernel_spmd(
        nc,
        in_maps,
        core_ids=list(range(N_CORES)),
        trace=_trace,
        tmpdir=_tmpdir,
    )

    Dt = _d_table(W2, b2).astype(np.float32)  # [P2, T]
    spk = np.empty((T, N_CORES * BC, H2), np.float32)
    mem = np.empty((T, N_CORES * BC, H2), np.float32)
    for c in range(N_CORES):
        r = res.results[c]
        mraw = r["mem_out"].astype(np.float32)  # [T, P2, HALF] = M2'
        sraw = r["sig_out"].astype(np.float32)  # [T, P2, HALF] = s2 (0/1)
        mfull = 0.5 * (mraw + Dt.T[:, :, None])  # mem2 = (M2' + D_t)/2
        for src, dst in ((mfull, mem), (sraw, spk)):
            q = src.reshape(T, 2, G, H2, HALF)  # [t, h, g, j, n]
            q = q.transpose(0, 2, 1, 4, 3)  # [t, g, h, n, j]
            dst[:, c * BC : (c + 1) * BC, :] = q.reshape(T, BC, H2)
    spk = (spk[:, :B_FULL, :] > 0.5).astype(np.float32)
    mem = mem[:, :B_FULL, :]
    kernel._last_results = res
    return spk, mem


# revision 36
# speedup vs baseline: 1.0376x; 1.0255x over previous
"""Trainium2 Bass kernel for a 2-layer spiking (LIF) MLP scan.

Model (per reference):
  cur1 = x @ W1.T + b1           [B, 25]   (constant over time)
  25 timesteps of:
    reset1 = H(mem1 - 1); mem1 = 0.95*mem1 + cur1 - reset1; spk1 = H(mem1 - 1)
    cur2 = spk1 @ W2.T + b2
    reset2 = H(mem2 - 1); mem2 = 0.95*mem2 + cur2 - reset2; spk2 = H(mem2 - 1)
  outputs: spk2_rec, mem2_rec  each [25, B, 10]

Distribution: pure data parallel over 8 NeuronCores; batch 32768 padded to
32800 = 8 cores x 4100.

Device formulation: doubled units M = 2*mem; layer-1 spikes sign-coded
sigma1 = 2*spk1 - 1 (ScalarE Sign, f16, feeds the W2 matmuls); layer-2
spikes 0/1-coded s2 = (M2' > 2 - D_t) via DVE is_gt:
    M1_t = beta*M1_{t-1} + chat - sigma1_{t-1},  chat = 2*cur1 - 1
    M2'_t = beta*M2'_{t-1} + (sigma1_t @ W2bd - 2*s2_{t-1})   [PSUM accum]
The layer-2 constant drive c = sum_k W2[j,k] + 2*b2[j] is removed from the
device recurrence; its accumulation D_t = beta*D_{t-1} + c is a host-side
per-(partition, step) table that shifts the is_gt threshold (2 - D_t). The
s2 reset-subtract is a PE matmul accumulate (lhsT = -2I) into the same
PSUM tile as the W2 drive. The raw f32 M2' ring is DMA'd out directly; the
host recovers mem2 = 0.5*(M2' + D_t) and spk2 = (s2 > 0.5).

Matmul precision: fp16 split-accumulate. x = xa + xb (two fp16 terms hold
~22 of f32's 24 mantissa bits), W = wa + wb likewise; accumulating
xa@wa + xa@wb + xb@wa in fp32 PSUM gives ~f32-class results at 1 cycle/row.
Spikes are exactly representable in fp16, so the layer-2 matmuls need only
the weight split (2 terms per batch half).

Per-core layout:
  - x arrives host-transposed as [784, 4100] fp16 split pairs; W1 is kept
    COMPACT: a zero-padded sliding-window strip [112, 7*245] per term, so
    each feature-packing group g gets a 125-wide lhsT window that places
    W1 at output partitions 25g:25(g+1) (PSUM base partition stays 0).
  - Layer-1 state is feature-packed: 5 batch groups x 25 features = 125
    partitions, 820 batch columns -> [125, 820] tiles.
  - Layer-2 state is batch-packed [100, 410]: row 50*h + 10*g + j holds
    feature j of batch element g*820 + h*410 + n.
  - The layer-1 recurrence runs as independent column chains: a DVE chain
    (stt + subtract, in-order on DVE) and a Pool chain (both ops in
    TensorScalarPtr form for gpsimd's better efficiency class). The Sign
    lives on ScalarE with one step of slack.
  - Emission order: GEMM-h0 -> h0 layer-1 front-run -> GEMM-h1 -> main
    loop, so ScalarE's h0 Signs are not queued behind the h1 PSUM copy
    (engines execute their streams in order).
"""

import numpy as np

BETA = 0.95
T = 25
B_FULL = 32768
D = 784
H1 = 25
H2 = 10
N_CORES = 8
BC = 4100          # per-core batch (padded)
G = 5              # feature-packing groups
COLS = BC // G     # 820 batch columns per group
HALF = COLS // 2   # 410
P1 = G * H1        # 125 partitions for layer-1 state
P2 = 2 * G * H2    # 100 rows of the layer-2 tile

KC = 7           # k-chunks of 112 rows: 7*112 = 784
KSZ = D // KC

# engine split knobs (tuned against TimelineSim)
POOL_H0 = 0        # h0 layer-1 subtract cols on gpsimd (rest DVE)
POOL_H1 = 288      # h1 layer-1 subtract cols on gpsimd (rest DVE)
SIG_RING = 25      # sigma1 tiles: one per step (full cross-step pipelining)
MH2_RING = 10      # layer-2 state/spike ring depth

_CACHED = {}


def _build_program(variant="v5"):
    from contextlib import ExitStack

    import concourse.bacc as bacc
    import concourse.tile as tile
    from concourse import mybir

    dt = mybir.dt
    alu = mybir.AluOpType
    act_fn = mybir.ActivationFunctionType

    nc = bacc.Bacc(
        "TRN2",
        target_bir_lowering=False,
        debug=False,
        enable_asserts=False,
        num_devices=N_CORES,
    )

    f16 = dt.float16
    f32 = dt.float32

    xa_d = nc.dram_tensor("xa", [D, BC], f16, kind="ExternalInput").ap()
    xb_d = nc.dram_tensor("xb", [D, BC], f16, kind="ExternalInput").ap()
    w1a_d = nc.dram_tensor("w1a", [D, H1], f16, kind="ExternalInput").ap()
    w1b_d = nc.dram_tensor("w1b", [D, H1], f16, kind="ExternalInput").ap()
    cb1_d = nc.dram_tensor("cb1", [P1, 1], f32, kind="ExternalInput").ap()
    # 4 block-diagonal W2 terms: [Aa, Ab, Ba, Bb] pairs with halves 0,0,1,1
    wbd_d = nc.dram_tensor("wbd", [P1, 4 * P2], f16, kind="ExternalInput").ap()
    negi_d = nc.dram_tensor("negi", [P2, P2], f16, kind="ExternalInput").ap()
    # per-step layer-2 threshold: 2 - D_t per partition
    t2tab_d = nc.dram_tensor("t2tab", [P2, T], f32, kind="ExternalInput").ap()

    mem_out = nc.dram_tensor(
        "mem_out", [T, P2, HALF], f32, kind="ExternalOutput"
    ).ap()
    sig_out = nc.dram_tensor(
        "sig_out", [T, P2, HALF], f16, kind="ExternalOutput"
    ).ap()

    with tile.TileContext(nc) as tc, ExitStack() as ctx:
        w1_pool = ctx.enter_context(tc.tile_pool(name="w1", bufs=1))
        wbd_pool = ctx.enter_context(tc.tile_pool(name="wbd", bufs=1))
        xin_pool = ctx.enter_context(tc.tile_pool(name="xin", bufs=10))
        state_pool = ctx.enter_context(tc.tile_pool(name="state", bufs=1))
        psA_pool = ctx.enter_context(tc.tile_pool(name="psA", bufs=2, space="PSUM"))
        ps2_pool = ctx.enter_context(tc.tile_pool(name="ps2", bufs=5, space="PSUM"))

        # --- weights: zero-padded sliding-window strip per split term ---
        WSTRIP = P1 + 120  # 245
        w1_tiles = []
        for src_d, tag in ((w1a_d, "w1a"), (w1b_d, "w1b")):
            wt = w1_pool.tile([KSZ, KC * WSTRIP], f16, tag=tag)
            nc.vector.memset(wt[:], 0.0)
            nc.scalar.dma_start(
                wt[:].rearrange("p (c w) -> p c w", c=KC)[:, :, 120 : 120 + H1],
                src_d.rearrange("(c p) n -> p c n", p=KSZ),
            )
            w1_tiles.append(wt)

        wbd_t = wbd_pool.tile([P1, 4 * P2], f16, tag="wbd")
        nc.scalar.dma_start(wbd_t[:], wbd_d[:])
        wbd_tiles = [wbd_t[:, i * P2 : (i + 1) * P2] for i in range(4)]
        negi = wbd_pool.tile([P2, P2], f16, tag="negi")
        nc.scalar.dma_start(negi[:], negi_d[:])
        t2tab = wbd_pool.tile([P2, T], f32, tag="t2tab")
        nc.scalar.dma_start(t2tab[:], t2tab_d[:])

        # --- persistent state ---
        chat1 = state_pool.tile([P1, COLS], f32, tag="chat1")
        mh1A = state_pool.tile([P1, COLS], f32, tag="mh1A")
        mh1B = state_pool.tile([P1, COLS], f32, tag="mh1B")
        sig_ring = []
        for i in range(SIG_RING):
            sg = state_pool.tile([P1, COLS], f16, tag=f"sig{i}")
            sig_ring.append(sg)
        sig_init = state_pool.tile([P1, COLS], f16, tag="sig_init")
        nc.vector.memset(sig_init[:], -1.0)
        nc.vector.memset(mh1B[:], 0.0)
        cb1 = state_pool.tile([P1, 1], f32, tag="cb1")
        nc.sync.dma_start(cb1[:], cb1_d[:])

        # layer-2 state/spike rings: one tile per slot
        mh2_ring = []
        s2_ring = []
        for i in range(MH2_RING):
            mh2slot = state_pool.tile([P2, HALF], f32, tag=f"mh2_{i}")
            mh2_ring.append(mh2slot[:])
            s2slot = state_pool.tile([P2, HALF], f16, tag=f"s2_{i}")
            s2_ring.append(s2slot[:])
        mh2_init = state_pool.tile([P2, HALF], f32, tag="mh2i")
        nc.vector.memset(mh2_init[:], 0.0)
        s2_init = state_pool.tile([P2, HALF], f16, tag="s2i")
        nc.vector.memset(s2_init[:], 0.0)

        # per-partition bias column (-2.0) for the layer-1 Sign activations
        biasc = state_pool.tile([128, 1], f32, tag="biasc")
        nc.vector.memset(biasc[:], -2.0)

        # --- main GEMM -> chat1 = 2*cur1 - 1, feature-packed [125, 820] ---
        def gemm_half(h):
            ps = psA_pool.tile([P1, HALF], f32)
            # issue all x DMAs in g-order, but consume groups rotated by one
            # so every tile has ~2 tile-times of DMA slack when the PE gets
            # to it (prevents p-state-resetting PE stalls at pair boundaries)
            gxts = []
            for g in range(G):
                col0 = g * COLS + h * HALF
                xts = []
                for xi, src_d in enumerate((xa_d, xb_d)):
                    xt = xin_pool.tile([KSZ, KC * HALF], f16, tag=f"x{xi}")
                    nc.sync.dma_start(
                        xt[:].rearrange("p (c n) -> p c n", c=KC),
                        src_d.rearrange("(c p) n -> p c n", p=KSZ)[
                            :, :, col0 : col0 + HALF
                        ],
                    )
                    xts.append(xt)
                gxts.append(xts)
            for gi, g in enumerate((1, 2, 3, 4, 0)):
                xts = gxts[g]
                terms = [(0, 0), (1, 0), (0, 1)]  # (w term, x term)
                for k in range(KC):
                    for j, (wi, xi) in enumerate(terms):
                        off = k * WSTRIP + 120 - H1 * g
                        nc.tensor.matmul(
                            ps[:],
                            lhsT=w1_tiles[wi][:, off : off + P1],
                            rhs=xts[xi][:, k * HALF : (k + 1) * HALF],
                            start=(gi == 0 and k == 0 and j == 0),
                            stop=(gi == G - 1 and k == KC - 1
                                  and j == len(terms) - 1),
                        )
            # bias (2*b1 - 1) folded into the PSUM->SBUF copy
            nc.scalar.activation(
                chat1[:, h * HALF : (h + 1) * HALF], ps[:],
                act_fn.Identity, bias=cb1[:], scale=1.0,
            )

        # --- the 25-step scan ---
        def l1_step(t, h, pool_cols):
            # stt on DVE (gpsimd cannot run scalar_tensor_tensor); the
            # sigma-subtract splits column-wise between DVE (in-order with
            # the stt) and Pool (tensor_tensor, the only gpsimd ALU form
            # walrus accepts)
            prev = sig_ring[t - 1] if t > 0 else sig_init
            new = sig_ring[t]
            m_prev = mh1B if t % 2 == 0 else mh1A
            m_cur = mh1A if t % 2 == 0 else mh1B
            sl = slice(h * HALF, (h + 1) * HALF)
            nc.vector.scalar_tensor_tensor(
                m_cur[:, sl], m_prev[:, sl], BETA, chat1[:, sl],
                op0=alu.mult, op1=alu.add,
            )
            dc = slice(h * HALF, (h + 1) * HALF - pool_cols)
            nc.vector.tensor_tensor(
                m_cur[:, dc], m_cur[:, dc], prev[:, dc], op=alu.subtract
            )
            if pool_cols:
                pc = slice((h + 1) * HALF - pool_cols, (h + 1) * HALF)
                nc.gpsimd.tensor_tensor(
                    m_cur[:, pc], m_cur[:, pc], prev[:, pc], op=alu.subtract
                )
            # sigma1 = Sign(M1 - 2)
            nc.scalar.activation(
                new[:, sl], m_cur[:, sl], act_fn.Sign,
                bias=biasc[0:P1, :], scale=1.0,
            )

        def l1_step_h0_split(t):
            # two independent 205-col sub-chains; per-block Sign shortens
            # the sub->Sign->sub cycle from ~1250ns to ~930ns
            prev = sig_ring[t - 1] if t > 0 else sig_init
            new = sig_ring[t]
            m_prev = mh1B if t % 2 == 0 else mh1A
            m_cur = mh1A if t % 2 == 0 else mh1B
            for b0, b1 in ((0, HALF // 2), (HALF // 2, HALF)):
                bc = slice(b0, b1)
                nc.vector.scalar_tensor_tensor(
                    m_cur[:, bc], m_prev[:, bc], BETA, chat1[:, bc],
                    op0=alu.mult, op1=alu.add,
                )
                nc.vector.tensor_tensor(
                    m_cur[:, bc], m_cur[:, bc], prev[:, bc], op=alu.subtract
                )
                nc.scalar.activation(
                    new[:, bc], m_cur[:, bc], act_fn.Sign,
                    bias=biasc[0:P1, :], scale=1.0,
                )

        gemm_half(0)
        for t in range(T):
            l1_step_h0_split(t)
        gemm_half(1)

        mh2_prev = mh2_init[:]
        s2_prev = s2_init[:]
        for t in range(T):
            l1_step(t, 1, POOL_H1)
            new = sig_ring[t]
            # layer-2 drive [100, 410]: blockdiag W2 terms + (-2I) s2 reset
            ps2 = ps2_pool.tile([P2, HALF], f32)
            mms = [(0, 0), (1, 0), (2, 1), (3, 1)]  # (wbd idx, half)
            for j, (wi, h) in enumerate(mms):
                nc.tensor.matmul(
                    ps2[:],
                    lhsT=wbd_tiles[wi],
                    rhs=new[:, h * HALF : (h + 1) * HALF],
                    start=(j == 0),
                    stop=False,
                )
            nc.tensor.matmul(
                ps2[:], lhsT=negi[:], rhs=s2_prev, start=False, stop=True
            )
            # M2' = beta*M2'_prev + (drive - 2*s2_prev)
            mh2n = mh2_ring[t % MH2_RING]
            nc.vector.scalar_tensor_tensor(
                mh2n, mh2_prev, BETA, ps2[:], op0=alu.mult, op1=alu.add
            )
            # s2 = (M2' > 2 - D_t)   (f16 0/1)
            s2n = s2_ring[t % MH2_RING]
            nc.vector.tensor_scalar(
                s2n, mh2n, t2tab[:, t : t + 1], None, op0=alu.is_gt
            )
            nc.sync.dma_start(mem_out[t], mh2n)
            nc.sync.dma_start(sig_out[t], s2n)
            mh2_prev = mh2n
            s2_prev = s2n

    nc.compile()
    return nc


def _get_nc(variant="v5"):
    key = (variant,)
    if key not in _CACHED:
        _CACHED[key] = _build_program(variant)
    return _CACHED[key]


def _f16_split(a):
    hi = a.astype(np.float16)
    lo = (a.astype(np.float32) - hi.astype(np.float32)).astype(np.float16)
    return hi, lo


def _d_table(W2, b2):
    # D_t = beta*D_{t-1} + c, c = sum_k W2[j,k] + 2*b2[j] per row j
    c = (W2.sum(axis=1) + 2.0 * b2).astype(np.float64)
    crow = np.tile(c, 2 * G)  # [P2] per partition 50h+10g+j
    Dt = np.zeros((P2, T), np.float64)
    acc = np.zeros(P2, np.float64)
    for t in range(T):
        acc = BETA * acc + crow
        Dt[:, t] = acc
    return Dt


def _host_inputs(x, W1, b1, W2, b2):
    ins = {}
    xp = np.zeros((D, N_CORES * BC), np.float32)
    xp[:, : x.shape[0]] = x.T
    ins["xa"], ins["xb"] = _f16_split(xp)
    w1t = 2.0 * W1.T.astype(np.float32)
    ins["w1a"], ins["w1b"] = _f16_split(w1t)
    ins["cb1"] = np.tile(2.0 * b1 - 1.0, G).astype(np.float32)[:, None]

    # chat2 = sigma1 @ blockdiag(W2.T); constant part goes via the D table
    wbdf = np.zeros((P1, 2 * P2), np.float32)
    for blk, h in ((0, 0), (1, 1)):
        off = blk * P2 + h * G * H2
        for g in range(G):
            wbdf[g * H1 : (g + 1) * H1, off + g * H2 : off + (g + 1) * H2] = W2.T
    A, B = wbdf[:, :P2], wbdf[:, P2:]
    Aa, Ab = _f16_split(A)
    Ba, Bb = _f16_split(B)
    ins["wbd"] = np.concatenate([Aa, Ab, Ba, Bb], axis=1)
    ins["negi"] = (-2.0 * np.eye(P2)).astype(np.float16)
    ins["t2tab"] = (2.0 - _d_table(W2, b2)).astype(np.float32)
    return ins


def kernel(x, W1, b1, W2, b2, _variant="v5", _trace=False, _tmpdir=None):
    from concourse.bass_utils import run_bass_kernel_spmd

    x = np.asarray(x, np.float32)
    W1 = np.asarray(W1, np.float32)
    b1 = np.asarray(b1, np.float32)
    W2 = np.asarray(W2, np.float32)
    b2 = np.asarray(b2, np.float32)
    B = x.shape[0]
    assert B == B_FULL, f"kernel hardcoded for B={B_FULL}, got {B}"

    full = _host_inputs(x, W1, b1, W2, b2)
    percore = ("xa", "xb")
    shared = {k: v for k, v in full.items() if k not in percore}
    in_maps = []
    for c in range(N_CORES):
        m = dict(shared)
        for k in percore:
            m[k] = np.ascontiguousarray(full[k][:, c * BC : (c + 1) * BC])
        in_maps.append(m)

    nc = _get_nc(_variant)
    res = run_bass_kernel_spmd(
        nc,
        in_maps,
        core_ids=list(range(N_CORES)),
        trace=_trace,
        tmpdir=_tmpdir,
    )

    Dt = _d_table(W2, b2).astype(np.float32)  # [P2, T]
    spk = np.empty((T, N_CORES * BC, H2), np.float32)
    mem = np.empty((T, N_CORES * BC, H2), np.float32)
    for c in range(N_CORES):
        r = res.results[c]
        mraw = r["mem_out"].astype(np.float32)  # [T, P2, HALF] = M2'
        sraw = r["sig_out"].astype(np.float32)  # [T, P2, HALF] = s2 (0/1)
        mfull = 0.5 * (mraw + Dt.T[:, :, None])  # mem2 = (M2' + D_t)/2
        for src, dst in ((mfull, mem), (sraw, spk)):
            q = src.reshape(T, 2, G, H2, HALF)  # [t, h, g, j, n]
            q = q.transpose(0, 2, 1, 4, 3)  # [t, g, h, n, j]
            dst[:, c * BC : (c + 1) * BC, :] = q.reshape(T, BC, H2)
    spk = (spk[:, :B_FULL, :] > 0.5).astype(np.float32)
    mem = mem[:, :B_FULL, :]
    kernel._last_results = res
    return spk, mem
